# revision 1
# baseline (speedup 1.0000x reference)
"""Chunked causal self-attention with RoPE — Trainium2 Bass/Tile kernel.

Problem: B=4, L=4096, H=16, Dh=Dv=128, chunked (C=1024) causal attention
with rotary embeddings, fp32 inputs/outputs.

Sharding: 8 cores = batch (4) x head-half (2). Each core handles one batch
element and 8 heads: 8 heads x 4 chunks = 32 independent chunk-attention
units of shape (1024, 1024).

Device algorithm per (head, chunk), everything fp16 on the PE:
  - Q/K arrive host-transposed as [dh partitions, l] (contraction dim on
    partitions) in a HEAD-PAIR layout: two heads' matching dh-halves share
    one 128-partition tile, so the RoPE half-mixing (d <-> d+64) is done
    with full-width DVE ops + per-head 64-partition combine writes — no
    duplicated bytes over HBM. K-side multiplies run on GPSIMD.
  - S^T[k, q] = K_r @ Q_r^T via PE (stationary = K^T tile, moving = Q^T),
    only the causal lower-triangle tiles. The diagonal 128x128 block's mask
    is folded into the PE as an extra accumulate-matmul adding
    -500 * max(0, k-q) (rank-128 ramp = strict-upper-ones @ scaled-ones).
  - P^T = exp(scale * S^T) on ACT (PSUM -> SBUF, fp16), one op per k-tile
    group ([0],[1,7],[2,6],[3,5],[4] share 2-bank PSUM tiles); no
    max-subtraction needed (scores are O(1) for randn-scale inputs).
  - O[q, dv] = sum_k P^T[k,q] V[k,dv] with a ones-column appended to V so
    column 128 of the PSUM accumulator is the softmax denominator.
  - normalize + evacuate: out = O[:, :128] * (1 / O[:, 128]) on DVE
    (1 of 8 q-tiles on ACT for balance), fp16 out, host upcasts to fp32.
"""

import functools
import math
import sys
from concurrent.futures import ThreadPoolExecutor

import numpy as np

if "/opt/trn_rl_repo" not in sys.path:
    sys.path.insert(0, "/opt/trn_rl_repo")

B, L, H, DH, DV = 4, 4096, 16, 128, 128
CHUNK = 1024
NCORES = 8
HPC = H // 2  # heads per core
NCH = L // CHUNK  # chunks
NT = CHUNK // 128  # 128-row tiles per chunk
ROPE_BASE = 10000.0
MASK_VAL = -500.0

_PROG_CACHE = {}


def _build_program(n_heads=HPC, n_chunks=NCH, repeat=1):
    """Build the (SPMD, per-core) Bass program. Parameterized so small
    configs can run in CoreSim quickly; repeat>1 duplicates the whole
    workload (for HW-time differencing benchmarks)."""
    from contextlib import ExitStack

    import concourse.bacc as bacc
    import concourse.tile as tile
    from concourse import mybir

    f16 = mybir.dt.float16
    f32 = mybir.dt.float32
    Lc = n_chunks * CHUNK
    scale = 1.0 / math.sqrt(DH)

    assert n_heads % 2 == 0, "head-pair layout needs an even head count"
    nh2 = n_heads // 2

    nc = bacc.Bacc(None, target_bir_lowering=False)
    # Head-pair layout: for pair j = (2j, 2j+1), dim 1 packs
    #   [qlo, qhi, klo, khi] where qlo = [head 2j dh 0:64 ; head 2j+1 dh 0:64]
    # (partition halves hold the SAME dh-half of the two heads), so RoPE's
    # half-mixing becomes within-op partition slices with no duplicated bytes.
    qk_d = nc.dram_tensor("qk", [nh2, 4, DH, Lc], f16, kind="ExternalInput")
    # va interleaves the pair per row: [hp, l, j, col]
    va_d = nc.dram_tensor("va", [nh2, Lc, 2, DV + 1], f16, kind="ExternalInput")
    cs_d = nc.dram_tensor("cs", [DH, Lc], f16, kind="ExternalInput")
    sn_d = nc.dram_tensor("sn", [DH, Lc], f16, kind="ExternalInput")
    at_d = nc.dram_tensor("at", [128, 128], f16, kind="ExternalInput")
    bm_d = nc.dram_tensor("bm", [128, 128], f16, kind="ExternalInput")
    o_d = nc.dram_tensor("o", [Lc, n_heads, DV], f16, kind="ExternalOutput")

    # out[l, h, dv] viewed as [hpair, p, chunk, qtile, j, dv]
    o_view = o_d.rearrange(
        "(cc qt p) (hp j) d -> hp p cc qt j d", qt=NT, p=128, j=2
    )

    # PSUM/P grouping: k-tiles grouped so each group's total width <= 1024
    # (2 PSUM banks): [0], [1,7], [2,6], [3,5], [4]. One exp per group.
    K_GROUPS = [(0,), (1, 7), (2, 6), (3, 5), (4,)]
    _off = {}
    _base = 0
    for _g in K_GROUPS:
        _o = _base
        for _ki in _g:
            _off[_ki] = _o
            _o += CHUNK - 128 * _ki
        _base += 1024
    P_COLS = 4608

    def p_off(ki):
        return _off[ki]

    with tile.TileContext(nc) as tc, ExitStack() as ctx:
        singles = ctx.enter_context(tc.tile_pool(name="singles", bufs=1))
        inp = ctx.enter_context(tc.tile_pool(name="inp", bufs=4))
        vp = ctx.enter_context(tc.tile_pool(name="vp", bufs=3))
        rp = ctx.enter_context(tc.tile_pool(name="rp", bufs=3))
        pp = ctx.enter_context(tc.tile_pool(name="pp", bufs=4))
        op = ctx.enter_context(tc.tile_pool(name="op", bufs=3))
        rcp = ctx.enter_context(tc.tile_pool(name="rcp", bufs=8))
        sps = ctx.enter_context(tc.tile_pool(name="sps", bufs=2, space="PSUM"))
        ops_ = ctx.enter_context(tc.tile_pool(name="ops", bufs=4, space="PSUM"))

        cs_t = singles.tile([DH, Lc], f16)
        nc.sync.dma_start(out=cs_t, in_=cs_d[:, :])
        sn_t = singles.tile([DH, Lc], f16)
        nc.sync.dma_start(out=sn_t, in_=sn_d[:, :])
        # ramp-mask factors: (at.T @ bm)[k, q] = MASK_VAL * max(0, k - q)
        at_t = singles.tile([128, 128], f16)
        nc.sync.dma_start(out=at_t, in_=at_d[:, :])
        bm_t = singles.tile([128, 128], f16)
        nc.sync.dma_start(out=bm_t, in_=bm_d[:, :])

        for h0 in range(nh2 * repeat):
            hp = h0 % nh2
            for c in range(n_chunks):
                c0 = c * CHUNK
                # split q/k loads into separate tiles so each rope path
                # starts on its own DMA arrival (per-tile dependency tracking)
                qk_src = qk_d[hp].rearrange("j p l -> p j l")
                qh_t = inp.tile([DH, 2, CHUNK], f16, tag="qh")
                nc.sync.dma_start(out=qh_t, in_=qk_src[:, 0:2, c0 : c0 + CHUNK])
                kh_t = inp.tile([DH, 2, CHUNK], f16, tag="kh")
                nc.sync.dma_start(out=kh_t, in_=qk_src[:, 2:4, c0 : c0 + CHUNK])
                va_t = vp.tile([128, NT, 2, DV + 1], f16, tag="va")
                nc.sync.dma_start(
                    out=va_t,
                    in_=va_d[hp].rearrange("(u p) j c -> p u j c", p=128)[
                        :, c * NT : (c + 1) * NT, :, :
                    ],
                )
                out_t = op.tile([128, NT, 2, DV], f16, tag="o")

                # RoPE for the head pair. For x in {q, k}:
                #   rot_j[0:64]   = xlo[64j:64j+64]*cos - xhi[64j:64j+64]*sin
                #   rot_j[64:128] = xhi[64j:64j+64]*cos + xlo[64j:64j+64]*sin
                csl = cs_t[:, c0 : c0 + CHUNK]
                snl = sn_t[:, c0 : c0 + CHUNK]
                qro = [
                    rp.tile([DH, CHUNK], f16, tag="qr0", name="qr0"),
                    rp.tile([DH, CHUNK], f16, tag="qr1", name="qr1"),
                ]
                kro = [
                    rp.tile([DH, CHUNK], f16, tag="kr0", name="kr0"),
                    rp.tile([DH, CHUNK], f16, tag="kr1", name="kr1"),
                ]
                t1 = rp.tile([DH, CHUNK], f16, tag="t1")
                t2 = rp.tile([DH, CHUNK], f16, tag="t2")
                t3 = rp.tile([DH, CHUNK], f16, tag="t3")
                t4 = rp.tile([DH, CHUNK], f16, tag="t4")
                nc.vector.tensor_mul(t1, qh_t[:, 0, :], csl)
                nc.vector.tensor_mul(t2, qh_t[:, 1, :], snl)
                nc.vector.tensor_mul(t3, qh_t[:, 1, :], csl)
                nc.vector.tensor_mul(t4, qh_t[:, 0, :], snl)
                for j in range(2):
                    s = slice(64 * j, 64 * j + 64)
                    nc.vector.tensor_sub(qro[j][0:64, :], t1[s, :], t2[s, :])
                    nc.vector.tensor_add(qro[j][64:128, :], t3[s, :], t4[s, :])
                # K multiplies on GPSIMD to offload the DVE
                u1 = rp.tile([DH, CHUNK], f16, tag="u1")
                u2 = rp.tile([DH, CHUNK], f16, tag="u2")
                u3 = rp.tile([DH, CHUNK], f16, tag="u3")
                u4 = rp.tile([DH, CHUNK], f16, tag="u4")
                nc.gpsimd.tensor_mul(u1, kh_t[:, 0, :], csl)
                nc.gpsimd.tensor_mul(u2, kh_t[:, 1, :], snl)
                nc.gpsimd.tensor_mul(u3, kh_t[:, 1, :], csl)
                nc.gpsimd.tensor_mul(u4, kh_t[:, 0, :], snl)
                for j in range(2):
                    s = slice(64 * j, 64 * j + 64)
                    nc.vector.tensor_sub(kro[j][0:64, :], u1[s, :], u2[s, :])
                    nc.vector.tensor_add(kro[j][64:128, :], u3[s, :], u4[s, :])

                for j in range(2):
                    qr = qro[j]
                    kr = kro[j]
                    # S^T phase, k-tiles grouped into <=1024-wide PSUM tiles.
                    # Diagonal 128-block gets a PE ramp mask accumulated in:
                    # exp(scale*s + scale*MASK_VAL*max(0, k-q)) -> 0 for k > q.
                    p_t = pp.tile([128, P_COLS], f16, tag="p")
                    for grp in K_GROUPS:
                        gw = sum(CHUNK - 128 * ki for ki in grp)
                        g0 = p_off(grp[0])
                        s_ps = sps.tile([128, gw], f32, tag="s")
                        for ki in grp:
                            off = p_off(ki) - g0
                            lhs = kr[:, ki * 128 : (ki + 1) * 128]
                            q0 = ki * 128
                            w = CHUNK - q0
                            # diagonal block: ramp mask then scores
                            nc.tensor.matmul(
                                s_ps[:, off : off + 128],
                                lhsT=at_t,
                                rhs=bm_t,
                                start=True,
                                stop=False,
                            )
                            nc.tensor.matmul(
                                s_ps[:, off : off + 128],
                                lhsT=lhs,
                                rhs=qr[:, q0 : q0 + 128],
                                start=False,
                                stop=True,
                            )
                            # rest: segments split at PSUM bank boundaries
                            seg = 128
                            while seg < w:
                                seg_end = min(
                                    w, ((off + seg) // 512 + 1) * 512 - off
                                )
                                nc.tensor.matmul(
                                    s_ps[:, off + seg : off + seg_end],
                                    lhsT=lhs,
                                    rhs=qr[:, q0 + seg : q0 + seg_end],
                                    start=True,
                                    stop=True,
                                )
                                seg = seg_end
                        nc.scalar.activation(
                            out=p_t[:, g0 : g0 + gw],
                            in_=s_ps[:, 0:gw],
                            func=mybir.ActivationFunctionType.Exp,
                            scale=scale,
                        )

                    # O phase: per q-tile, accumulate over k-tiles <= q-tile
                    for qi in range(NT):
                        o_ps = ops_.tile([128, DV + 1], f32, tag="ops")
                        for ki in range(qi + 1):
                            pk = p_off(ki) + 128 * (qi - ki)
                            nc.tensor.matmul(
                                o_ps,
                                lhsT=p_t[:, pk : pk + 128],
                                rhs=va_t[:, ki, j, :],
                                start=(ki == 0),
                                stop=(ki == qi),
                            )
                        rc = rcp.tile([128, 1], f32, tag="rc")
                        nc.vector.reciprocal(rc, o_ps[:, DV : DV + 1])
                        if qi == 4:
                            # one of eight evacs rides on ACT to balance DVE
                            nc.scalar.activation(
                                out=out_t[:, qi, j, :],
                                in_=o_ps[:, 0:DV],
                                func=mybir.ActivationFunctionType.Copy,
                                scale=rc[:],
                            )
                        else:
                            nc.vector.tensor_scalar_mul(
                                out_t[:, qi, j, :], in0=o_ps[:, 0:DV], scalar1=rc
                            )
                nc.sync.dma_start(out=o_view[hp, :, c], in_=out_t)

    nc.finalize()
    return nc


def _get_program(n_heads=HPC, n_chunks=NCH, repeat=1):
    key = (n_heads, n_chunks, repeat)
    if key not in _PROG_CACHE:
        _PROG_CACHE[key] = _build_program(n_heads, n_chunks, repeat)
    return _PROG_CACHE[key]


@functools.lru_cache(maxsize=4)
def _rope_tables(start_index, Lc):
    half = DH // 2
    freqs = np.exp(np.arange(half, dtype=np.float64) * -(math.log(ROPE_BASE) / half))
    ang = (np.arange(Lc, dtype=np.float64) + float(start_index))[None, :] * freqs[:, None]
    cos = np.cos(ang)
    sin = np.sin(ang)
    cs2 = np.concatenate([cos, cos], axis=0).astype(np.float16)  # [128, Lc]
    sn2 = np.concatenate([sin, sin], axis=0).astype(np.float16)  # [128, Lc]
    return cs2, sn2


@functools.lru_cache(maxsize=1)
def _mask_consts():
    j = np.arange(128)
    at = (j[:, None] < j[None, :]).astype(np.float16)  # AT[j, k] = 1 if j < k
    bm = np.where(j[:, None] >= j[None, :], np.float16(MASK_VAL), np.float16(0.0))
    return np.ascontiguousarray(at), np.ascontiguousarray(bm)


def _prep_core(q, k, v, start_index, b, hh, n_heads=HPC, n_chunks=NCH):
    """Build one core's input map from full fp32 inputs (head-pair layout)."""
    Lc = n_chunks * CHUNK
    nh2 = n_heads // 2
    qt = np.ascontiguousarray(
        q[b, :Lc, hh : hh + n_heads, :].transpose(1, 2, 0), dtype=np.float16
    )  # (nh, 128, Lc)
    kt = np.ascontiguousarray(
        k[b, :Lc, hh : hh + n_heads, :].transpose(1, 2, 0), dtype=np.float16
    )
    # pair layout: lo = [head 2j dh 0:64 ; head 2j+1 dh 0:64], hi likewise
    qp = qt.reshape(nh2, 2, 2, 64, Lc)  # (pair, head-in-pair, dh-half, 64, Lc)
    kp = kt.reshape(nh2, 2, 2, 64, Lc)
    qlo = qp[:, :, 0].reshape(nh2, 128, Lc)
    qhi = qp[:, :, 1].reshape(nh2, 128, Lc)
    klo = kp[:, :, 0].reshape(nh2, 128, Lc)
    khi = kp[:, :, 1].reshape(nh2, 128, Lc)
    qk = np.ascontiguousarray(np.stack([qlo, qhi, klo, khi], axis=1))
    vv = v[b, :Lc, hh : hh + n_heads, :].transpose(1, 0, 2).astype(np.float16)
    va = np.concatenate([vv, np.ones((n_heads, Lc, 1), np.float16)], axis=2)
    # interleave the pair per row: (nh2, Lc, 2, DV+1)
    va = np.ascontiguousarray(
        va.reshape(nh2, 2, Lc, DV + 1).transpose(0, 2, 1, 3)
    )
    cs2, sn2 = _rope_tables(start_index, Lc)
    at, bm = _mask_consts()
    return {
        "qk": qk,
        "va": va,
        "cs": np.ascontiguousarray(cs2),
        "sn": np.ascontiguousarray(sn2),
        "at": at,
        "bm": bm,
    }


def _run(in_maps, n_heads=HPC, n_chunks=NCH, trace=False):
    from concourse.bass_utils import run_bass_kernel_spmd

    nc = _get_program(n_heads, n_chunks)
    return run_bass_kernel_spmd(
        nc, in_maps, core_ids=list(range(len(in_maps))), trace=trace
    )


def kernel(q, k, v, start_index):
    q = np.asarray(q, dtype=np.float32)
    k = np.asarray(k, dtype=np.float32)
    v = np.asarray(v, dtype=np.float32)
    si = float(np.asarray(start_index))

    # shard prep is pure numpy copies/casts (GIL-released) — parallelize
    with ThreadPoolExecutor(max_workers=NCORES) as ex:
        in_maps = list(
            ex.map(
                lambda core: _prep_core(
                    q, k, v, si, core // 2, (core % 2) * HPC
                ),
                range(NCORES),
            )
        )

    res = _run(in_maps)

    out = np.empty((B, L, H, DV), np.float32)
    for core in range(NCORES):
        b = core // 2
        hh = (core % 2) * HPC
        out[b, :, hh : hh + HPC, :] = res.results[core]["o"].astype(np.float32)
    return out.reshape(B, L, H * DV)



# revision 15
# speedup vs baseline: 1.1536x; 1.1536x over previous
"""Chunked causal self-attention with RoPE — Trainium2 Bass/Tile kernel.

Problem: B=4, L=4096, H=16, Dh=Dv=128, chunked (C=1024) causal attention
with rotary embeddings, fp32 inputs/outputs.

Sharding: 8 cores = batch (4) x head-half (2). Each core handles one batch
element and 8 heads: 8 heads x 4 chunks = 32 independent chunk-attention
units of shape (1024, 1024).

Device algorithm per (head, chunk) unit, fp16 on the PE:
  - Q/K arrive host-transposed as [dh partitions, l] per head, TWICE: the
    natural order [q1; q2] and the half-swapped order [q2; q1].  With a
    sign-folded sin table ssn = [-sin; +sin], RoPE collapses to three
    full-width (128-partition) DVE/GPSIMD ops per side:
        rot = q * [cos; cos] + qswap * [-sin; sin]
    (no 64-partition half-combines; the extra q/k copies ride on DMA
    bandwidth, which has slack).
  - S^T[k, q] = K_r @ Q_r^T via PE, causal lower-triangle tiles only.  The
    diagonal 128x128 block's mask is folded into the PE as an extra
    accumulate-matmul adding -500 * max(0, k-q).
  - k-tiles are packed into 3 PSUM groups of exactly 1536 columns
    ({0,4}, {1,3}, {2,5,6,7}), so exp(scale * S^T) runs as only 3 wide
    ACT ops per unit (the ACT engine is the throughput floor of the whole
    kernel: its cost is free-size * 0.83ns regardless of dtype).
  - O[q, dv] = sum_k P^T[k,q] V[k,dv] with a ones-column appended to V so
    column 128 of the PSUM accumulator is the softmax denominator.  O
    accumulators for 3 q-tiles share one PSUM bank ([128, 3, 129] fp32).
  - normalize + evacuate: one DVE reciprocal per pack ([128,3] strided from
    the denominator columns), then a single DVE tensor_mul over the whole
    pack with the reciprocal broadcast along the free dim (stride-0 AP).
  - The O-phase of unit u is emitted after the S-phase of unit u+1 so the
    PE never stalls waiting for the last exp of its own unit.
Engine balance per core (cost-model): DMA ~146us total over 3 queues,
ACT ~142us (exp floor), PE ~141us, DVE ~137us, Pool ~128us; wall ~183us.
"""

import functools
import math
import sys
from concurrent.futures import ThreadPoolExecutor

import numpy as np

if "/opt/trn_rl_repo" not in sys.path:
    sys.path.insert(0, "/opt/trn_rl_repo")

B, L, H, DH, DV = 4, 4096, 16, 128, 128
CHUNK = 1024
NCORES = 8
HPC = H // 2  # heads per core
NCH = L // CHUNK  # chunks
NT = CHUNK // 128  # 128-row tiles per chunk
ROPE_BASE = 10000.0
MASK_VAL = -500.0

# k-tile widths are CHUNK - 128*ki; these groups each sum to exactly 1536
# columns = 3 PSUM banks, so exp is 3 wide ACT ops per (head, chunk).
K_GROUPS = [(0, 4), (1, 3), (2, 5, 6, 7)]
GROUP_W = 1536
# q-tile packs sharing one [128, 3, 129] fp32 PSUM accumulator (1 bank).
Q_PACKS = [(0, 1, 2), (3, 4, 5), (6, 7)]
# 5 of every 8 units (head, chunk) run their k-side rope on GPSIMD (Pool)
# instead of DVE, balancing the two engines; interleaved so neither engine
# becomes the per-unit bottleneck for a long stretch.
POOL_KROPE_MOD = (0, 2, 3, 5, 7)

_PROG_CACHE = {}


def _p_offsets():
    """Column offsets of each k-tile inside the P tile ([128, 4608] SBUF)
    and inside its group's PSUM tile."""
    p_off, ps_off = {}, {}
    g0 = 0
    for grp in K_GROUPS:
        o = 0
        for ki in grp:
            p_off[ki] = g0 + o
            ps_off[ki] = o
            o += CHUNK - 128 * ki
        assert o == GROUP_W
        g0 += GROUP_W
    return p_off, ps_off


P_OFF, PS_OFF = _p_offsets()
P_COLS = GROUP_W * len(K_GROUPS)


def _build_program(n_heads=HPC, n_chunks=NCH):
    """Build the (SPMD, per-core) Bass program. Parameterized so small
    configs can run in CoreSim quickly."""
    from contextlib import ExitStack

    import concourse.bacc as bacc
    import concourse.tile as tile
    from concourse import mybir

    f16 = mybir.dt.float16
    f32 = mybir.dt.float32
    Lc = n_chunks * CHUNK
    scale = 1.0 / math.sqrt(DH)
    nkt = n_chunks * NT  # k-tiles over the whole length

    nc = bacc.Bacc(None, target_bir_lowering=False)
    # dim1: 0=q, 1=q half-swapped, 2=k, 3=k half-swapped
    qk_d = nc.dram_tensor("qk", [n_heads, 4, DH, Lc], f16, kind="ExternalInput")
    # V with ones column, partition-major so DMA runs are 4KB contiguous
    va_d = nc.dram_tensor("va", [n_heads, 128, nkt, DV + 1], f16, kind="ExternalInput")
    cs_d = nc.dram_tensor("cs", [DH, Lc], f16, kind="ExternalInput")
    sn_d = nc.dram_tensor("sn", [DH, Lc], f16, kind="ExternalInput")  # [-sin; +sin]
    at_d = nc.dram_tensor("at", [128, 128], f16, kind="ExternalInput")
    bm_d = nc.dram_tensor("bm", [128, 128], f16, kind="ExternalInput")
    # output, q-tile pairs interleaved so DMA runs are 512B
    nqp = (n_chunks * NT + 1) // 2
    o_d = nc.dram_tensor("o", [n_heads, nqp, 128, 2, DV], f16, kind="ExternalOutput")

    with tile.TileContext(nc) as tc, ExitStack() as ctx:
        singles = ctx.enter_context(tc.tile_pool(name="singles", bufs=1))
        inp = ctx.enter_context(tc.tile_pool(name="inp", bufs=5))
        vp = ctx.enter_context(tc.tile_pool(name="vp", bufs=2))
        rp = ctx.enter_context(tc.tile_pool(name="rp", bufs=2))
        pp = ctx.enter_context(tc.tile_pool(name="pp", bufs=3))
        op = ctx.enter_context(tc.tile_pool(name="op", bufs=3))
        rcp = ctx.enter_context(tc.tile_pool(name="rcp", bufs=6))
        sps = ctx.enter_context(tc.tile_pool(name="sps", bufs=2, space="PSUM"))
        ops_ = ctx.enter_context(tc.tile_pool(name="ops", bufs=2, space="PSUM"))

        cs_t = singles.tile([DH, Lc], f16)
        nc.gpsimd.dma_start(out=cs_t, in_=cs_d[:, :])
        sn_t = singles.tile([DH, Lc], f16)
        nc.scalar.dma_start(out=sn_t, in_=sn_d[:, :])
        # ramp-mask factors: (at.T @ bm)[k, q] = MASK_VAL * max(0, k - q)
        at_t = singles.tile([128, 128], f16)
        nc.sync.dma_start(out=at_t, in_=at_d[:, :])
        bm_t = singles.tile([128, 128], f16)
        nc.sync.dma_start(out=bm_t, in_=bm_d[:, :])

        def o_phase(h, c, p_t, va_t, out_t):
            """Emit the O-phase + normalize for unit (h, c)."""
            def omms(pack):
                om = ops_.tile([128, len(Q_PACKS[0]), DV + 1], f32, tag="om", name="om")
                for t, qi in enumerate(pack):
                    for ki in range(qi + 1):
                        pcol = P_OFF[ki] + 128 * (qi - ki)
                        nc.tensor.matmul(
                            om[:, t, :],
                            lhsT=p_t[:, pcol : pcol + 128],
                            rhs=va_t[:, c * NT + ki, :],
                            start=(ki == 0),
                            stop=(ki == qi),
                        )
                return om

            def onorm(pack, om):
                npk = len(pack)
                rc = rcp.tile([128, len(Q_PACKS[0])], f32, tag="rc", name="rc")
                nc.vector.reciprocal(rc[:, 0:npk], om[:, 0:npk, DV])
                rb = rc[:, 0:npk, None].broadcast_to([128, npk, DV])
                q0 = pack[0]
                nc.vector.tensor_mul(
                    out_t[:, q0 : q0 + npk, :], om[:, 0:npk, 0:DV], rb
                )

            om0 = omms(Q_PACKS[0])
            om1 = omms(Q_PACKS[1])
            onorm(Q_PACKS[0], om0)
            om2 = omms(Q_PACKS[2])
            onorm(Q_PACKS[1], om1)
            onorm(Q_PACKS[2], om2)
            # output DMA: q-tile pairs interleaved (512B runs on the DRAM side)
            qt0 = c * NT
            oeng = nc.scalar if (h * n_chunks + c) % 2 == 0 else nc.sync
            oeng.dma_start(
                out=o_d[h, qt0 // 2 : (qt0 + NT) // 2].rearrange(
                    "qp p two d -> p qp two d"
                ),
                in_=out_t.rearrange("p (qp two) d -> p qp two d", two=2),
            )

        pending = None
        for h in range(n_heads):
            va_t = vp.tile([128, nkt, DV + 1], f16, tag="va", name="va")
            for c in range(n_chunks):
                c0 = c * CHUNK
                qk_t = inp.tile([DH, 4, CHUNK], f16, tag="qk", name="qk")
                nc.sync.dma_start(
                    out=qk_t,
                    in_=qk_d[h].rearrange("j p l -> p j l")[:, :, c0 : c0 + CHUNK],
                )
                if c == 0:
                    # va is first needed by the O-phase, well after this
                    # chunk's qk — keep it off the prefetch critical path
                    nc.sync.dma_start(out=va_t, in_=va_d[h])

                csl = cs_t[:, c0 : c0 + CHUNK]
                snl = sn_t[:, c0 : c0 + CHUNK]
                unit = h * n_chunks + c
                keng = nc.gpsimd if unit % 8 in POOL_KROPE_MOD else nc.vector

                t1 = rp.tile([DH, CHUNK], f16, tag="t1", name="t1")
                t2 = rp.tile([DH, CHUNK], f16, tag="t2", name="t2")
                qr = rp.tile([DH, CHUNK], f16, tag="qr", name="qr")
                nc.vector.tensor_mul(t1, qk_t[:, 0, :], csl)
                nc.vector.tensor_mul(t2, qk_t[:, 1, :], snl)
                nc.vector.tensor_add(qr, t1, t2)
                u1 = rp.tile([DH, CHUNK], f16, tag="u1", name="u1")
                u2 = rp.tile([DH, CHUNK], f16, tag="u2", name="u2")
                kr = rp.tile([DH, CHUNK], f16, tag="kr", name="kr")
                keng.tensor_mul(u1, qk_t[:, 2, :], csl)
                keng.tensor_mul(u2, qk_t[:, 3, :], snl)
                keng.tensor_add(kr, u1, u2)

                # S^T phase: k-tiles in 3 groups of 1536 cols (3 PSUM banks)
                p_t = pp.tile([128, P_COLS], f16, tag="p", name="p")
                for gi, grp in enumerate(K_GROUPS):
                    s_ps = sps.tile([128, GROUP_W], f32, tag="s", name="s")
                    for ki in grp:
                        off = PS_OFF[ki]
                        lhs = kr[:, ki * 128 : (ki + 1) * 128]
                        q0 = ki * 128
                        w = CHUNK - q0
                        # diagonal block: ramp mask then scores
                        nc.tensor.matmul(
                            s_ps[:, off : off + 128],
                            lhsT=at_t,
                            rhs=bm_t,
                            start=True,
                            stop=False,
                        )
                        nc.tensor.matmul(
                            s_ps[:, off : off + 128],
                            lhsT=lhs,
                            rhs=qr[:, q0 : q0 + 128],
                            start=False,
                            stop=True,
                        )
                        # rest: segments split at PSUM bank boundaries
                        seg = 128
                        while seg < w:
                            seg_end = min(w, ((off + seg) // 512 + 1) * 512 - off)
                            nc.tensor.matmul(
                                s_ps[:, off + seg : off + seg_end],
                                lhsT=lhs,
                                rhs=qr[:, q0 + seg : q0 + seg_end],
                                start=True,
                                stop=True,
                            )
                            seg = seg_end
                    nc.scalar.activation(
                        out=p_t[:, gi * GROUP_W : (gi + 1) * GROUP_W],
                        in_=s_ps,
                        func=mybir.ActivationFunctionType.Exp,
                        scale=scale,
                    )

                out_t = op.tile([128, NT, DV], f16, tag="o", name="o")
                if pending is not None:
                    o_phase(*pending)
                pending = (h, c, p_t, va_t, out_t)
        o_phase(*pending)

    nc.finalize()
    return nc


def _get_program(n_heads=HPC, n_chunks=NCH):
    key = (n_heads, n_chunks)
    if key not in _PROG_CACHE:
        _PROG_CACHE[key] = _build_program(n_heads, n_chunks)
    return _PROG_CACHE[key]


@functools.lru_cache(maxsize=4)
def _rope_tables(start_index, Lc):
    half = DH // 2
    freqs = np.exp(np.arange(half, dtype=np.float64) * -(math.log(ROPE_BASE) / half))
    ang = (np.arange(Lc, dtype=np.float64) + float(start_index))[None, :] * freqs[:, None]
    cos = np.cos(ang)
    sin = np.sin(ang)
    cs2 = np.concatenate([cos, cos], axis=0).astype(np.float16)  # [128, Lc]
    ssn = np.concatenate([-sin, sin], axis=0).astype(np.float16)  # [128, Lc], sign-folded
    return cs2, ssn


@functools.lru_cache(maxsize=1)
def _mask_consts():
    j = np.arange(128)
    at = (j[:, None] < j[None, :]).astype(np.float16)  # AT[j, k] = 1 if j < k
    bm = np.where(j[:, None] >= j[None, :], np.float16(MASK_VAL), np.float16(0.0))
    return np.ascontiguousarray(at), np.ascontiguousarray(bm)


def _prep_core(q, k, v, start_index, b, hh, n_heads=HPC, n_chunks=NCH):
    """Build one core's input map from full fp32 inputs."""
    Lc = n_chunks * CHUNK
    nkt = n_chunks * NT
    qt = np.ascontiguousarray(
        q[b, :Lc, hh : hh + n_heads, :].transpose(1, 2, 0), dtype=np.float16
    )  # (nh, 128, Lc)
    kt = np.ascontiguousarray(
        k[b, :Lc, hh : hh + n_heads, :].transpose(1, 2, 0), dtype=np.float16
    )
    half = DH // 2
    qk = np.empty((n_heads, 4, DH, Lc), np.float16)
    qk[:, 0] = qt
    qk[:, 1, 0:half] = qt[:, half:DH]
    qk[:, 1, half:DH] = qt[:, 0:half]
    qk[:, 2] = kt
    qk[:, 3, 0:half] = kt[:, half:DH]
    qk[:, 3, half:DH] = kt[:, 0:half]

    vv = v[b, :Lc, hh : hh + n_heads, :].transpose(1, 0, 2).astype(np.float16)
    va = np.concatenate([vv, np.ones((n_heads, Lc, 1), np.float16)], axis=2)
    # partition-major: [h, 128, nkt, DV+1]
    va = np.ascontiguousarray(va.reshape(n_heads, nkt, 128, DV + 1).transpose(0, 2, 1, 3))
    cs2, ssn = _rope_tables(start_index, Lc)
    at, bm = _mask_consts()
    return {
        "qk": np.ascontiguousarray(qk),
        "va": va,
        "cs": np.ascontiguousarray(cs2),
        "sn": np.ascontiguousarray(ssn),
        "at": at,
        "bm": bm,
    }


def _unshard_core(o, n_heads=HPC, n_chunks=NCH):
    """o: [n_heads, nqp, 128, 2, DV] -> [Lc, n_heads, DV] fp32."""
    Lc = n_chunks * CHUNK
    # position l = (qp*2 + two)*128 + p
    return (
        o.transpose(1, 3, 2, 0, 4).reshape(Lc, n_heads, DV).astype(np.float32)
    )


def _run(in_maps, n_heads=HPC, n_chunks=NCH, trace=False):
    from concourse.bass_utils import run_bass_kernel_spmd

    nc = _get_program(n_heads, n_chunks)
    return run_bass_kernel_spmd(
        nc, in_maps, core_ids=list(range(len(in_maps))), trace=trace
    )


def kernel(q, k, v, start_index):
    q = np.asarray(q, dtype=np.float32)
    k = np.asarray(k, dtype=np.float32)
    v = np.asarray(v, dtype=np.float32)
    si = float(np.asarray(start_index))

    # shard prep is pure numpy copies/casts (GIL-released) — parallelize
    with ThreadPoolExecutor(max_workers=NCORES) as ex:
        in_maps = list(
            ex.map(
                lambda core: _prep_core(q, k, v, si, core // 2, (core % 2) * HPC),
                range(NCORES),
            )
        )

    res = _run(in_maps)

    out = np.empty((B, L, H, DV), np.float32)
    for core in range(NCORES):
        b = core // 2
        hh = (core % 2) * HPC
        out[b, :, hh : hh + HPC, :] = _unshard_core(res.results[core]["o"])
    return out.reshape(B, L, H * DV)


# revision 18
# speedup vs baseline: 1.1847x; 1.0269x over previous
"""Chunked causal self-attention with RoPE — Trainium2 Bass/Tile kernel.

Problem: B=4, L=4096, H=16, Dh=Dv=128, chunked (C=1024) causal attention
with rotary embeddings, fp32 inputs/outputs.

Sharding: 8 cores = batch (4) x head-half (2). Each core handles one batch
element and 8 heads: 8 heads x 4 chunks = 32 independent chunk-attention
units of shape (1024, 1024).

Device algorithm per (head, chunk) unit, fp16 on the PE:
  - Q/K arrive host-transposed as [dh partitions, l] per head, TWICE: the
    natural order [q1; q2] and the half-swapped order [q2; q1].  With a
    sign-folded sin table ssn = [-sin; +sin], RoPE collapses to three
    full-width (128-partition) DVE/GPSIMD ops per side:
        rot = q * [cos; cos] + qswap * [-sin; sin]
    (no 64-partition half-combines; the extra q/k copies ride on DMA
    bandwidth, which has slack).
  - S^T[k, q] = K_r @ Q_r^T via PE, causal lower-triangle tiles only.  The
    diagonal 128x128 block's mask is folded into the PE as an extra
    accumulate-matmul adding -500 * max(0, k-q).
  - k-tiles are packed into 3 PSUM groups of exactly 1536 columns
    ({0,4}, {1,3}, {2,5,6,7}), so exp(scale * S^T) runs as only 3 wide
    ACT ops per unit (the ACT engine is the throughput floor of the whole
    kernel: its cost is free-size * 0.83ns regardless of dtype).
  - O[q, dv] = sum_k P^T[k,q] V[k,dv] with a ones-column appended to V so
    column 128 of the PSUM accumulator is the softmax denominator.  O
    accumulators for 3 q-tiles share one PSUM bank ([128, 3, 129] fp32).
  - normalize + evacuate: one DVE reciprocal per pack ([128,3] strided from
    the denominator columns), then a single DVE tensor_mul over the whole
    pack with the reciprocal broadcast along the free dim (stride-0 AP).
  - The O-phase of unit u is emitted after the S-phase of unit u+1 so the
    PE never stalls waiting for the last exp of its own unit.
Engine balance per core (cost-model): DMA ~146us total over 3 queues,
ACT ~142us (exp floor), PE ~141us, DVE ~137us, Pool ~128us; wall ~183us.
"""

import functools
import math
import sys
from concurrent.futures import ThreadPoolExecutor

import numpy as np

if "/opt/trn_rl_repo" not in sys.path:
    sys.path.insert(0, "/opt/trn_rl_repo")

B, L, H, DH, DV = 4, 4096, 16, 128, 128
CHUNK = 1024
NCORES = 8
HPC = H // 2  # heads per core
NCH = L // CHUNK  # chunks
NT = CHUNK // 128  # 128-row tiles per chunk
ROPE_BASE = 10000.0
MASK_VAL = -500.0

# k-tile widths are CHUNK - 128*ki; these groups each sum to exactly 1536
# columns = 3 PSUM banks, so exp is 3 wide ACT ops per (head, chunk).
K_GROUPS = [(0, 4), (1, 3), (2, 5, 6, 7)]
GROUP_W = 1536
# q-tile packs sharing one [128, 3, 129] fp32 PSUM accumulator (1 bank).
Q_PACKS = [(0, 1, 2), (3, 4, 5), (6, 7)]
# 5 of every 8 units (head, chunk) run their k-side rope on GPSIMD (Pool)
# instead of DVE, balancing the two engines; interleaved so neither engine
# becomes the per-unit bottleneck for a long stretch.
POOL_KROPE_MOD = (0, 2, 3, 5, 7)

_PROG_CACHE = {}


def _p_offsets():
    """Column offsets of each k-tile inside the P tile ([128, 4608] SBUF)
    and inside its group's PSUM tile."""
    p_off, ps_off = {}, {}
    g0 = 0
    for grp in K_GROUPS:
        o = 0
        for ki in grp:
            p_off[ki] = g0 + o
            ps_off[ki] = o
            o += CHUNK - 128 * ki
        assert o == GROUP_W
        g0 += GROUP_W
    return p_off, ps_off


P_OFF, PS_OFF = _p_offsets()
P_COLS = GROUP_W * len(K_GROUPS)


def _build_program(n_heads=HPC, n_chunks=NCH):
    """Build the (SPMD, per-core) Bass program. Parameterized so small
    configs can run in CoreSim quickly."""
    from contextlib import ExitStack

    import concourse.bacc as bacc
    import concourse.tile as tile
    from concourse import mybir

    f16 = mybir.dt.float16
    f32 = mybir.dt.float32
    Lc = n_chunks * CHUNK
    scale = 1.0 / math.sqrt(DH)
    nkt = n_chunks * NT  # k-tiles over the whole length

    nc = bacc.Bacc(None, target_bir_lowering=False)
    # dim1: 0=q, 1=q half-swapped, 2=k, 3=k half-swapped
    qk_d = nc.dram_tensor("qk", [n_heads, 4, DH, Lc], f16, kind="ExternalInput")
    # V with ones column, partition-major so DMA runs are 4KB contiguous
    va_d = nc.dram_tensor("va", [n_heads, 128, nkt, DV + 1], f16, kind="ExternalInput")
    cs_d = nc.dram_tensor("cs", [DH, CHUNK], f16, kind="ExternalInput")
    sn_d = nc.dram_tensor("sn", [DH, CHUNK], f16, kind="ExternalInput")  # [-sin; +sin]
    at_d = nc.dram_tensor("at", [128, 128], f16, kind="ExternalInput")
    bm_d = nc.dram_tensor("bm", [128, 128], f16, kind="ExternalInput")
    # output, q-tile pairs interleaved so DMA runs are 512B
    nqp = (n_chunks * NT + 1) // 2
    o_d = nc.dram_tensor("o", [n_heads, nqp, 128, 2, DV], f16, kind="ExternalOutput")

    with tile.TileContext(nc) as tc, ExitStack() as ctx:
        singles = ctx.enter_context(tc.tile_pool(name="singles", bufs=1))
        inp = ctx.enter_context(tc.tile_pool(name="inp", bufs=5))
        vp = ctx.enter_context(tc.tile_pool(name="vp", bufs=2))
        rp = ctx.enter_context(tc.tile_pool(name="rp", bufs=2))
        pp = ctx.enter_context(tc.tile_pool(name="pp", bufs=3))
        op = ctx.enter_context(tc.tile_pool(name="op", bufs=3))
        rcp = ctx.enter_context(tc.tile_pool(name="rcp", bufs=6))
        sps = ctx.enter_context(tc.tile_pool(name="sps", bufs=2, space="PSUM"))
        ops_ = ctx.enter_context(tc.tile_pool(name="ops", bufs=2, space="PSUM"))

        cs_t = singles.tile([DH, CHUNK], f16)
        nc.gpsimd.dma_start(out=cs_t, in_=cs_d[:, :])
        sn_t = singles.tile([DH, CHUNK], f16)
        nc.scalar.dma_start(out=sn_t, in_=sn_d[:, :])
        # ramp-mask factors: (at.T @ bm)[k, q] = MASK_VAL * max(0, k - q)
        at_t = singles.tile([128, 128], f16)
        nc.sync.dma_start(out=at_t, in_=at_d[:, :])
        bm_t = singles.tile([128, 128], f16)
        nc.sync.dma_start(out=bm_t, in_=bm_d[:, :])

        def o_phase(h, c, p_t, va_t, out_t):
            """Emit the O-phase + normalize for unit (h, c)."""
            def omms(pack):
                om = ops_.tile([128, len(Q_PACKS[0]), DV + 1], f32, tag="om", name="om")
                for t, qi in enumerate(pack):
                    for ki in range(qi + 1):
                        pcol = P_OFF[ki] + 128 * (qi - ki)
                        nc.tensor.matmul(
                            om[:, t, :],
                            lhsT=p_t[:, pcol : pcol + 128],
                            rhs=va_t[:, c * NT + ki, :],
                            start=(ki == 0),
                            stop=(ki == qi),
                        )
                return om

            def onorm(pack, om):
                npk = len(pack)
                rc = rcp.tile([128, len(Q_PACKS[0])], f32, tag="rc", name="rc")
                nc.vector.reciprocal(rc[:, 0:npk], om[:, 0:npk, DV])
                rb = rc[:, 0:npk, None].broadcast_to([128, npk, DV])
                q0 = pack[0]
                nc.vector.tensor_mul(
                    out_t[:, q0 : q0 + npk, :], om[:, 0:npk, 0:DV], rb
                )

            om0 = omms(Q_PACKS[0])
            om1 = omms(Q_PACKS[1])
            onorm(Q_PACKS[0], om0)
            om2 = omms(Q_PACKS[2])
            onorm(Q_PACKS[1], om1)
            onorm(Q_PACKS[2], om2)
            # output DMA: q-tile pairs interleaved (512B runs on the DRAM side)
            qt0 = c * NT
            oeng = nc.scalar if (h * n_chunks + c) % 2 == 0 else nc.sync
            oeng.dma_start(
                out=o_d[h, qt0 // 2 : (qt0 + NT) // 2].rearrange(
                    "qp p two d -> p qp two d"
                ),
                in_=out_t.rearrange("p (qp two) d -> p qp two d", two=2),
            )

        pending = None
        for h in range(n_heads):
            va_t = vp.tile([128, nkt, DV + 1], f16, tag="va", name="va")
            for c in range(n_chunks):
                c0 = c * CHUNK
                qk_t = inp.tile([DH, 4, CHUNK], f16, tag="qk", name="qk")
                nc.sync.dma_start(
                    out=qk_t,
                    in_=qk_d[h].rearrange("j p l -> p j l")[:, :, c0 : c0 + CHUNK],
                )
                if c == 0:
                    # va is first needed by the O-phase, well after this
                    # chunk's qk — keep it off the prefetch critical path
                    nc.sync.dma_start(out=va_t, in_=va_d[h])

                csl = cs_t[:, 0:CHUNK]
                snl = sn_t[:, 0:CHUNK]
                unit = h * n_chunks + c
                keng = nc.gpsimd if unit % 8 in POOL_KROPE_MOD else nc.vector

                t1 = rp.tile([DH, CHUNK], f16, tag="t1", name="t1")
                t2 = rp.tile([DH, CHUNK], f16, tag="t2", name="t2")
                qr = rp.tile([DH, CHUNK], f16, tag="qr", name="qr")
                nc.vector.tensor_mul(t1, qk_t[:, 0, :], csl)
                nc.vector.tensor_mul(t2, qk_t[:, 1, :], snl)
                nc.vector.tensor_add(qr, t1, t2)
                u1 = rp.tile([DH, CHUNK], f16, tag="u1", name="u1")
                u2 = rp.tile([DH, CHUNK], f16, tag="u2", name="u2")
                kr = rp.tile([DH, CHUNK], f16, tag="kr", name="kr")
                keng.tensor_mul(u1, qk_t[:, 2, :], csl)
                keng.tensor_mul(u2, qk_t[:, 3, :], snl)
                keng.tensor_add(kr, u1, u2)

                # S^T phase: k-tiles in 3 groups of 1536 cols (3 PSUM banks)
                p_t = pp.tile([128, P_COLS], f16, tag="p", name="p")
                for gi, grp in enumerate(K_GROUPS):
                    s_ps = sps.tile([128, GROUP_W], f32, tag="s", name="s")
                    for ki in grp:
                        off = PS_OFF[ki]
                        lhs = kr[:, ki * 128 : (ki + 1) * 128]
                        q0 = ki * 128
                        w = CHUNK - q0
                        # diagonal block: ramp mask then scores
                        nc.tensor.matmul(
                            s_ps[:, off : off + 128],
                            lhsT=at_t,
                            rhs=bm_t,
                            start=True,
                            stop=False,
                        )
                        nc.tensor.matmul(
                            s_ps[:, off : off + 128],
                            lhsT=lhs,
                            rhs=qr[:, q0 : q0 + 128],
                            start=False,
                            stop=True,
                        )
                        # rest: segments split at PSUM bank boundaries
                        seg = 128
                        while seg < w:
                            seg_end = min(w, ((off + seg) // 512 + 1) * 512 - off)
                            nc.tensor.matmul(
                                s_ps[:, off + seg : off + seg_end],
                                lhsT=lhs,
                                rhs=qr[:, q0 + seg : q0 + seg_end],
                                start=True,
                                stop=True,
                            )
                            seg = seg_end
                    nc.scalar.activation(
                        out=p_t[:, gi * GROUP_W : (gi + 1) * GROUP_W],
                        in_=s_ps,
                        func=mybir.ActivationFunctionType.Exp,
                        scale=scale,
                    )

                out_t = op.tile([128, NT, DV], f16, tag="o", name="o")
                if pending is not None:
                    o_phase(*pending)
                pending = (h, c, p_t, va_t, out_t)
        o_phase(*pending)

    nc.finalize()
    return nc


def _get_program(n_heads=HPC, n_chunks=NCH):
    key = (n_heads, n_chunks)
    if key not in _PROG_CACHE:
        _PROG_CACHE[key] = _build_program(n_heads, n_chunks)
    return _PROG_CACHE[key]


@functools.lru_cache(maxsize=4)
def _rope_tables(start_index, Lc):
    half = DH // 2
    freqs = np.exp(np.arange(half, dtype=np.float64) * -(math.log(ROPE_BASE) / half))
    ang = (np.arange(Lc, dtype=np.float64) + float(start_index))[None, :] * freqs[:, None]
    cos = np.cos(ang)
    sin = np.sin(ang)
    cs2 = np.concatenate([cos, cos], axis=0).astype(np.float16)  # [128, Lc]
    ssn = np.concatenate([-sin, sin], axis=0).astype(np.float16)  # [128, Lc], sign-folded
    return cs2, ssn


@functools.lru_cache(maxsize=1)
def _mask_consts():
    j = np.arange(128)
    at = (j[:, None] < j[None, :]).astype(np.float16)  # AT[j, k] = 1 if j < k
    bm = np.where(j[:, None] >= j[None, :], np.float16(MASK_VAL), np.float16(0.0))
    return np.ascontiguousarray(at), np.ascontiguousarray(bm)


def _prep_core(q, k, v, start_index, b, hh, n_heads=HPC, n_chunks=NCH):
    """Build one core's input map from full fp32 inputs."""
    Lc = n_chunks * CHUNK
    nkt = n_chunks * NT
    qt = np.ascontiguousarray(
        q[b, :Lc, hh : hh + n_heads, :].transpose(1, 2, 0), dtype=np.float16
    )  # (nh, 128, Lc)
    kt = np.ascontiguousarray(
        k[b, :Lc, hh : hh + n_heads, :].transpose(1, 2, 0), dtype=np.float16
    )
    half = DH // 2
    qk = np.empty((n_heads, 4, DH, Lc), np.float16)
    qk[:, 0] = qt
    qk[:, 1, 0:half] = qt[:, half:DH]
    qk[:, 1, half:DH] = qt[:, 0:half]
    qk[:, 2] = kt
    qk[:, 3, 0:half] = kt[:, half:DH]
    qk[:, 3, half:DH] = kt[:, 0:half]

    vv = v[b, :Lc, hh : hh + n_heads, :].transpose(1, 0, 2).astype(np.float16)
    va = np.concatenate([vv, np.ones((n_heads, Lc, 1), np.float16)], axis=2)
    # partition-major: [h, 128, nkt, DV+1]
    va = np.ascontiguousarray(va.reshape(n_heads, nkt, 128, DV + 1).transpose(0, 2, 1, 3))
    cs2, ssn = _rope_tables(start_index, CHUNK)
    at, bm = _mask_consts()
    return {
        "qk": np.ascontiguousarray(qk),
        "va": va,
        "cs": np.ascontiguousarray(cs2),
        "sn": np.ascontiguousarray(ssn),
        "at": at,
        "bm": bm,
    }


def _unshard_core(o, n_heads=HPC, n_chunks=NCH):
    """o: [n_heads, nqp, 128, 2, DV] -> [Lc, n_heads, DV] fp32."""
    Lc = n_chunks * CHUNK
    # position l = (qp*2 + two)*128 + p
    return (
        o.transpose(1, 3, 2, 0, 4).reshape(Lc, n_heads, DV).astype(np.float32)
    )


def _run(in_maps, n_heads=HPC, n_chunks=NCH, trace=False):
    from concourse.bass_utils import run_bass_kernel_spmd

    nc = _get_program(n_heads, n_chunks)
    return run_bass_kernel_spmd(
        nc, in_maps, core_ids=list(range(len(in_maps))), trace=trace
    )


def kernel(q, k, v, start_index):
    q = np.asarray(q, dtype=np.float32)
    k = np.asarray(k, dtype=np.float32)
    v = np.asarray(v, dtype=np.float32)
    si = float(np.asarray(start_index))

    # shard prep is pure numpy copies/casts (GIL-released) — parallelize
    with ThreadPoolExecutor(max_workers=NCORES) as ex:
        in_maps = list(
            ex.map(
                lambda core: _prep_core(q, k, v, si, core // 2, (core % 2) * HPC),
                range(NCORES),
            )
        )

    res = _run(in_maps)

    out = np.empty((B, L, H, DV), np.float32)
    for core in range(NCORES):
        b = core // 2
        hh = (core % 2) * HPC
        out[b, :, hh : hh + HPC, :] = _unshard_core(res.results[core]["o"])
    return out.reshape(B, L, H * DV)


# revision 20
# speedup vs baseline: 1.1865x; 1.0015x over previous
"""Chunked causal self-attention with RoPE — Trainium2 Bass/Tile kernel.

Problem: B=4, L=4096, H=16, Dh=Dv=128, chunked (C=1024) causal attention
with rotary embeddings, fp32 inputs/outputs.

Sharding: 8 cores = batch (4) x head-half (2). Each core handles one batch
element and 8 heads: 8 heads x 4 chunks = 32 independent chunk-attention
units of shape (1024, 1024).

Device algorithm per (head, chunk) unit, fp16 on the PE:
  - Q/K arrive host-transposed as [dh partitions, l] per head, TWICE: the
    natural order [q1; q2] and the half-swapped order [q2; q1].  With a
    sign-folded sin table ssn = [-sin; +sin], RoPE collapses to three
    full-width (128-partition) DVE/GPSIMD ops per side:
        rot = q * [cos; cos] + qswap * [-sin; sin]
    (no 64-partition half-combines; the extra q/k copies ride on DMA
    bandwidth, which has slack).
  - S^T[k, q] = K_r @ Q_r^T via PE, causal lower-triangle tiles only.  The
    diagonal 128x128 block's mask is folded into the PE as an extra
    accumulate-matmul adding -500 * max(0, k-q).
  - k-tiles are packed into 3 PSUM groups of exactly 1536 columns
    ({0,4}, {1,3}, {2,5,6,7}), so exp(scale * S^T) runs as only 3 wide
    ACT ops per unit (the ACT engine is the throughput floor of the whole
    kernel: its cost is free-size * 0.83ns regardless of dtype).
  - O[q, dv] = sum_k P^T[k,q] V[k,dv] with a ones-column appended to V so
    column 128 of the PSUM accumulator is the softmax denominator.  O
    accumulators for 3 q-tiles share one PSUM bank ([128, 3, 129] fp32).
  - normalize + evacuate: one DVE reciprocal per pack ([128,3] strided from
    the denominator columns), then a single DVE tensor_mul over the whole
    pack with the reciprocal broadcast along the free dim (stride-0 AP).
  - The O-phase of unit u is emitted after the S-phase of unit u+1 so the
    PE never stalls waiting for the last exp of its own unit.
Engine balance per core (cost-model): DMA ~142us total over 3 queues,
ACT ~142us (exp floor), PE ~141us, DVE ~137us, Pool ~128us; wall ~178us.
RoPE tables are chunk-local (1024 cols): chunk scores depend only on
position differences, so every chunk shares the positions-0..1023 table.
"""

import functools
import math
import sys
from concurrent.futures import ThreadPoolExecutor

import numpy as np

if "/opt/trn_rl_repo" not in sys.path:
    sys.path.insert(0, "/opt/trn_rl_repo")

B, L, H, DH, DV = 4, 4096, 16, 128, 128
CHUNK = 1024
NCORES = 8
HPC = H // 2  # heads per core
NCH = L // CHUNK  # chunks
NT = CHUNK // 128  # 128-row tiles per chunk
ROPE_BASE = 10000.0
MASK_VAL = -500.0

# k-tile widths are CHUNK - 128*ki; these groups each sum to exactly 1536
# columns = 3 PSUM banks, so exp is 3 wide ACT ops per (head, chunk).
K_GROUPS = [(0, 4), (1, 3), (2, 5, 6, 7)]
GROUP_W = 1536
# q-tile packs sharing one [128, 3, 129] fp32 PSUM accumulator (1 bank).
Q_PACKS = [(0, 1, 2), (3, 4, 5), (6, 7)]
# 5 of every 8 units (head, chunk) run their k-side rope on GPSIMD (Pool)
# instead of DVE, balancing the two engines; interleaved so neither engine
# becomes the per-unit bottleneck for a long stretch.
POOL_KROPE_MOD = (0, 2, 3, 5, 7)

_PROG_CACHE = {}


def _p_offsets():
    """Column offsets of each k-tile inside the P tile ([128, 4608] SBUF)
    and inside its group's PSUM tile."""
    p_off, ps_off = {}, {}
    g0 = 0
    for grp in K_GROUPS:
        o = 0
        for ki in grp:
            p_off[ki] = g0 + o
            ps_off[ki] = o
            o += CHUNK - 128 * ki
        assert o == GROUP_W
        g0 += GROUP_W
    return p_off, ps_off


P_OFF, PS_OFF = _p_offsets()
P_COLS = GROUP_W * len(K_GROUPS)


def _build_program(n_heads=HPC, n_chunks=NCH):
    """Build the (SPMD, per-core) Bass program. Parameterized so small
    configs can run in CoreSim quickly."""
    from contextlib import ExitStack

    import concourse.bacc as bacc
    import concourse.tile as tile
    from concourse import mybir

    f16 = mybir.dt.float16
    f32 = mybir.dt.float32
    Lc = n_chunks * CHUNK
    scale = 1.0 / math.sqrt(DH)
    nkt = n_chunks * NT  # k-tiles over the whole length

    nc = bacc.Bacc(None, target_bir_lowering=False)
    # dim1: 0=q, 1=q half-swapped, 2=k, 3=k half-swapped
    qk_d = nc.dram_tensor("qk", [n_heads, 4, DH, Lc], f16, kind="ExternalInput")
    # V with ones column, partition-major so DMA runs are 4KB contiguous
    va_d = nc.dram_tensor("va", [n_heads, 128, nkt, DV + 1], f16, kind="ExternalInput")
    cs_d = nc.dram_tensor("cs", [DH, CHUNK], f16, kind="ExternalInput")
    sn_d = nc.dram_tensor("sn", [DH, CHUNK], f16, kind="ExternalInput")  # [-sin; +sin]
    at_d = nc.dram_tensor("at", [128, 128], f16, kind="ExternalInput")
    bm_d = nc.dram_tensor("bm", [128, 128], f16, kind="ExternalInput")
    # output, q-tile pairs interleaved so DMA runs are 512B
    nqp = (n_chunks * NT + 1) // 2
    o_d = nc.dram_tensor("o", [n_heads, nqp, 128, 2, DV], f16, kind="ExternalOutput")

    with tile.TileContext(nc) as tc, ExitStack() as ctx:
        singles = ctx.enter_context(tc.tile_pool(name="singles", bufs=1))
        inp = ctx.enter_context(tc.tile_pool(name="inp", bufs=7))
        vp = ctx.enter_context(tc.tile_pool(name="vp", bufs=3))
        rp = ctx.enter_context(tc.tile_pool(name="rp", bufs=2))
        pp = ctx.enter_context(tc.tile_pool(name="pp", bufs=3))
        op = ctx.enter_context(tc.tile_pool(name="op", bufs=3))
        rcp = ctx.enter_context(tc.tile_pool(name="rcp", bufs=6))
        sps = ctx.enter_context(tc.tile_pool(name="sps", bufs=2, space="PSUM"))
        ops_ = ctx.enter_context(tc.tile_pool(name="ops", bufs=2, space="PSUM"))

        cs_t = singles.tile([DH, CHUNK], f16)
        nc.gpsimd.dma_start(out=cs_t, in_=cs_d[:, :])
        sn_t = singles.tile([DH, CHUNK], f16)
        nc.scalar.dma_start(out=sn_t, in_=sn_d[:, :])
        # ramp-mask factors: (at.T @ bm)[k, q] = MASK_VAL * max(0, k - q)
        at_t = singles.tile([128, 128], f16)
        nc.sync.dma_start(out=at_t, in_=at_d[:, :])
        bm_t = singles.tile([128, 128], f16)
        nc.sync.dma_start(out=bm_t, in_=bm_d[:, :])

        def o_phase(h, c, p_t, va_t, out_t):
            """Emit the O-phase + normalize for unit (h, c)."""
            def omms(pack):
                om = ops_.tile([128, len(Q_PACKS[0]), DV + 1], f32, tag="om", name="om")
                for t, qi in enumerate(pack):
                    for ki in range(qi + 1):
                        pcol = P_OFF[ki] + 128 * (qi - ki)
                        nc.tensor.matmul(
                            om[:, t, :],
                            lhsT=p_t[:, pcol : pcol + 128],
                            rhs=va_t[:, c * NT + ki, :],
                            start=(ki == 0),
                            stop=(ki == qi),
                        )
                return om

            def onorm(pack, om):
                npk = len(pack)
                rc = rcp.tile([128, len(Q_PACKS[0])], f32, tag="rc", name="rc")
                nc.vector.reciprocal(rc[:, 0:npk], om[:, 0:npk, DV])
                rb = rc[:, 0:npk, None].broadcast_to([128, npk, DV])
                q0 = pack[0]
                nc.vector.tensor_mul(
                    out_t[:, q0 : q0 + npk, :], om[:, 0:npk, 0:DV], rb
                )

            om0 = omms(Q_PACKS[0])
            om1 = omms(Q_PACKS[1])
            onorm(Q_PACKS[0], om0)
            om2 = omms(Q_PACKS[2])
            onorm(Q_PACKS[1], om1)
            onorm(Q_PACKS[2], om2)
            # output DMA: q-tile pairs interleaved (512B runs on the DRAM side)
            qt0 = c * NT
            oeng = nc.scalar if (h * n_chunks + c) % 2 == 0 else nc.sync
            oeng.dma_start(
                out=o_d[h, qt0 // 2 : (qt0 + NT) // 2].rearrange(
                    "qp p two d -> p qp two d"
                ),
                in_=out_t.rearrange("p (qp two) d -> p qp two d", two=2),
            )

        pending = None
        for h in range(n_heads):
            va_t = vp.tile([128, nkt, DV + 1], f16, tag="va", name="va")
            for c in range(n_chunks):
                c0 = c * CHUNK
                qk_t = inp.tile([DH, 4, CHUNK], f16, tag="qk", name="qk")
                nc.sync.dma_start(
                    out=qk_t,
                    in_=qk_d[h].rearrange("j p l -> p j l")[:, :, c0 : c0 + CHUNK],
                )
                if c == 0:
                    # va is first needed by the O-phase, well after this
                    # chunk's qk — keep it off the prefetch critical path
                    nc.sync.dma_start(out=va_t, in_=va_d[h])

                csl = cs_t[:, 0:CHUNK]
                snl = sn_t[:, 0:CHUNK]
                unit = h * n_chunks + c
                keng = nc.gpsimd if unit % 8 in POOL_KROPE_MOD else nc.vector

                t1 = rp.tile([DH, CHUNK], f16, tag="t1", name="t1")
                t2 = rp.tile([DH, CHUNK], f16, tag="t2", name="t2")
                qr = rp.tile([DH, CHUNK], f16, tag="qr", name="qr")
                nc.vector.tensor_mul(t1, qk_t[:, 0, :], csl)
                nc.vector.tensor_mul(t2, qk_t[:, 1, :], snl)
                nc.vector.tensor_add(qr, t1, t2)
                u1 = rp.tile([DH, CHUNK], f16, tag="u1", name="u1")
                u2 = rp.tile([DH, CHUNK], f16, tag="u2", name="u2")
                kr = rp.tile([DH, CHUNK], f16, tag="kr", name="kr")
                keng.tensor_mul(u1, qk_t[:, 2, :], csl)
                keng.tensor_mul(u2, qk_t[:, 3, :], snl)
                keng.tensor_add(kr, u1, u2)

                # S^T phase: k-tiles in 3 groups of 1536 cols (3 PSUM banks)
                p_t = pp.tile([128, P_COLS], f16, tag="p", name="p")
                for gi, grp in enumerate(K_GROUPS):
                    s_ps = sps.tile([128, GROUP_W], f32, tag="s", name="s")
                    for ki in grp:
                        off = PS_OFF[ki]
                        lhs = kr[:, ki * 128 : (ki + 1) * 128]
                        q0 = ki * 128
                        w = CHUNK - q0
                        # diagonal block: ramp mask then scores
                        nc.tensor.matmul(
                            s_ps[:, off : off + 128],
                            lhsT=at_t,
                            rhs=bm_t,
                            start=True,
                            stop=False,
                        )
                        nc.tensor.matmul(
                            s_ps[:, off : off + 128],
                            lhsT=lhs,
                            rhs=qr[:, q0 : q0 + 128],
                            start=False,
                            stop=True,
                        )
                        # rest: segments split at PSUM bank boundaries
                        seg = 128
                        while seg < w:
                            seg_end = min(w, ((off + seg) // 512 + 1) * 512 - off)
                            nc.tensor.matmul(
                                s_ps[:, off + seg : off + seg_end],
                                lhsT=lhs,
                                rhs=qr[:, q0 + seg : q0 + seg_end],
                                start=True,
                                stop=True,
                            )
                            seg = seg_end
                    nc.scalar.activation(
                        out=p_t[:, gi * GROUP_W : (gi + 1) * GROUP_W],
                        in_=s_ps,
                        func=mybir.ActivationFunctionType.Exp,
                        scale=scale,
                    )

                out_t = op.tile([128, NT, DV], f16, tag="o", name="o")
                if pending is not None:
                    o_phase(*pending)
                pending = (h, c, p_t, va_t, out_t)
        o_phase(*pending)

    nc.finalize()
    return nc


def _get_program(n_heads=HPC, n_chunks=NCH):
    key = (n_heads, n_chunks)
    if key not in _PROG_CACHE:
        _PROG_CACHE[key] = _build_program(n_heads, n_chunks)
    return _PROG_CACHE[key]


@functools.lru_cache(maxsize=4)
def _rope_tables(start_index, Lc):
    half = DH // 2
    freqs = np.exp(np.arange(half, dtype=np.float64) * -(math.log(ROPE_BASE) / half))
    ang = (np.arange(Lc, dtype=np.float64) + float(start_index))[None, :] * freqs[:, None]
    cos = np.cos(ang)
    sin = np.sin(ang)
    cs2 = np.concatenate([cos, cos], axis=0).astype(np.float16)  # [128, Lc]
    ssn = np.concatenate([-sin, sin], axis=0).astype(np.float16)  # [128, Lc], sign-folded
    return cs2, ssn


@functools.lru_cache(maxsize=1)
def _mask_consts():
    j = np.arange(128)
    at = (j[:, None] < j[None, :]).astype(np.float16)  # AT[j, k] = 1 if j < k
    bm = np.where(j[:, None] >= j[None, :], np.float16(MASK_VAL), np.float16(0.0))
    return np.ascontiguousarray(at), np.ascontiguousarray(bm)


def _prep_core(q, k, v, start_index, b, hh, n_heads=HPC, n_chunks=NCH):
    """Build one core's input map from full fp32 inputs."""
    Lc = n_chunks * CHUNK
    nkt = n_chunks * NT
    qt = np.ascontiguousarray(
        q[b, :Lc, hh : hh + n_heads, :].transpose(1, 2, 0), dtype=np.float16
    )  # (nh, 128, Lc)
    kt = np.ascontiguousarray(
        k[b, :Lc, hh : hh + n_heads, :].transpose(1, 2, 0), dtype=np.float16
    )
    half = DH // 2
    qk = np.empty((n_heads, 4, DH, Lc), np.float16)
    qk[:, 0] = qt
    qk[:, 1, 0:half] = qt[:, half:DH]
    qk[:, 1, half:DH] = qt[:, 0:half]
    qk[:, 2] = kt
    qk[:, 3, 0:half] = kt[:, half:DH]
    qk[:, 3, half:DH] = kt[:, 0:half]

    vv = v[b, :Lc, hh : hh + n_heads, :].transpose(1, 0, 2).astype(np.float16)
    va = np.concatenate([vv, np.ones((n_heads, Lc, 1), np.float16)], axis=2)
    # partition-major: [h, 128, nkt, DV+1]
    va = np.ascontiguousarray(va.reshape(n_heads, nkt, 128, DV + 1).transpose(0, 2, 1, 3))
    cs2, ssn = _rope_tables(start_index, CHUNK)
    at, bm = _mask_consts()
    return {
        "qk": np.ascontiguousarray(qk),
        "va": va,
        "cs": np.ascontiguousarray(cs2),
        "sn": np.ascontiguousarray(ssn),
        "at": at,
        "bm": bm,
    }


def _unshard_core(o, n_heads=HPC, n_chunks=NCH):
    """o: [n_heads, nqp, 128, 2, DV] -> [Lc, n_heads, DV] fp32."""
    Lc = n_chunks * CHUNK
    # position l = (qp*2 + two)*128 + p
    return (
        o.transpose(1, 3, 2, 0, 4).reshape(Lc, n_heads, DV).astype(np.float32)
    )


def _run(in_maps, n_heads=HPC, n_chunks=NCH, trace=False):
    from concourse.bass_utils import run_bass_kernel_spmd

    nc = _get_program(n_heads, n_chunks)
    return run_bass_kernel_spmd(
        nc, in_maps, core_ids=list(range(len(in_maps))), trace=trace
    )


def kernel(q, k, v, start_index):
    q = np.asarray(q, dtype=np.float32)
    k = np.asarray(k, dtype=np.float32)
    v = np.asarray(v, dtype=np.float32)
    si = float(np.asarray(start_index))

    # shard prep is pure numpy copies/casts (GIL-released) — parallelize
    with ThreadPoolExecutor(max_workers=NCORES) as ex:
        in_maps = list(
            ex.map(
                lambda core: _prep_core(q, k, v, si, core // 2, (core % 2) * HPC),
                range(NCORES),
            )
        )

    res = _run(in_maps)

    out = np.empty((B, L, H, DV), np.float32)
    for core in range(NCORES):
        b = core // 2
        hh = (core % 2) * HPC
        out[b, :, hh : hh + HPC, :] = _unshard_core(res.results[core]["o"])
    return out.reshape(B, L, H * DV)
